# revision 18
# baseline (speedup 1.0000x reference)
"""GAT (graph attention) kernel for 8 Trainium2 NeuronCores.

Strategy (1D dst-partitioning + host-side halo expansion):
  * Core k owns dst nodes [k*npc, (k+1)*npc).  Host appends self-loops and
    buckets edges by (dst core, dst chunk of 128), padding each chunk's
    bucket to a multiple of 128 edges with uniform tile counts across cores,
    so ONE SPMD program serves all 8 cores.
  * Instead of an on-device gather of source-node features (SWDGE
    descriptor generation is ~8ns/edge, a 1.8ms serial floor on GpSimd),
    the HOST expands the halo: xE[slot] = x[src[slot]] in edge-slot order.
    Each core streams its xE slabs contiguously and recomputes
    h_e = xE_e @ [W | w_src] on the tensor engine per edge slot (PE has
    headroom; 4.4x redundant h-flops beat the descriptor-gen wall).
  * Host also ships the per-tile one-hot scatter matrices (fp16 0/1):
    mask4[e,d] (edge->dst) and its transpose maskT[d,e] — DMA is cheaper
    than building them with iota/is_equal on the DVE.
  * Device, per dst chunk c (128 dst nodes), per 128-edge tile:
      hp   = xE_tile @ [W | w_src]          (PSUM, fp16 matmul)
      aep  = maskT @ a_dst_chunk            (per-edge dst logit)
      e    = hp[:,256:260] + aep; expE = exp(leaky_relu(e))
      brhs = [h * expE | expE]              (fp16)
      out_ps[d] += mask4^T @ brhs           (PSUM chain over the chunk:
                                             numerator + denominator)
    a_dst_chunk comes from a tiny precompute pass over the core's own
    x rows (4-column matmul).  Softmax division, bias, relu, L2-normalize
    on the chunk tail.  exp() skips the segment-max shift: logits are O(10)
    so exp stays in range, and softmax is shift-invariant.
  * PSUM->SBUF fp16 casts run on the (otherwise idle) GpSimd engine to
    keep the DVE off the critical path.
"""

import sys

sys.path.insert(0, "/opt/trn_rl_repo")

import numpy as np

HEADS = 4
OUT_CH = 64
NEG_SLOPE = 0.2
P = 128


# --------------------------------------------------------------------------
# host-side preprocessing (sharding + layout only, plus parameter fusion)
# --------------------------------------------------------------------------
def _preprocess(x, edge_index, W, att_src, att_dst, bias, n_cores):
    x = np.asarray(x, np.float32)
    N, IN = x.shape
    assert N % n_cores == 0
    npc = N // n_cores
    chunks = (npc + P - 1) // P

    src = np.concatenate(
        [np.asarray(edge_index[0], np.int64), np.arange(N, dtype=np.int64)]
    )
    dst = np.concatenate(
        [np.asarray(edge_index[1], np.int64), np.arange(N, dtype=np.int64)]
    )

    core = dst // npc
    rem = dst - core * npc
    chunk = rem // P
    dstl = (rem - chunk * P).astype(np.float32)

    # per-core edge buckets by dst chunk
    per_core = []
    for k in range(n_cores):
        sel = np.nonzero(core == k)[0]
        key = chunk[sel]
        order = np.argsort(key, kind="stable")
        counts = np.bincount(key, minlength=chunks)
        starts = np.zeros(chunks + 1, np.int64)
        np.cumsum(counts, out=starts[1:])
        per_core.append((src[sel][order], dstl[sel][order], counts, starts))

    all_counts = np.stack([pc[2] for pc in per_core])  # [cores, chunks]
    Tch = np.maximum(1, -(-all_counts.max(axis=0) // P))  # [chunks]
    slots_per_chunk = P * Tch
    total_slots = int(slots_per_chunk.sum())
    TT = int(total_slots // P)

    chunk_off = np.zeros(chunks + 1, np.int64)
    np.cumsum(slots_per_chunk, out=chunk_off[1:])

    src_pad = np.zeros((n_cores, total_slots), np.int64)
    dstl_pad = np.full((n_cores, total_slots), -1.0, np.float32)
    for k in range(n_cores):
        src_s, dstl_s, counts, starts = per_core[k]
        for c in range(chunks):
            off = int(chunk_off[c])
            s0, s1 = int(starts[c]), int(starts[c + 1])
            n = s1 - s0
            src_pad[k, off : off + n] = src_s[s0:s1]
            dstl_pad[k, off : off + n] = dstl_s[s0:s1]

    # host-built one-hot scatter matrices (fp16 0/1):
    #   mask4[e, t, d] = 1 if edge slot (t*128+e) targets local dst d
    #   maskT[d, t, e] = transpose of the same
    lane = np.arange(P, dtype=np.float32)
    arr = dstl_pad.reshape(n_cores, TT, P)  # [k, t, e]
    m4 = np.empty((n_cores, P, TT, P), np.float16)
    mT = np.empty((n_cores, P, TT, P), np.float16)
    for k in range(n_cores):
        eq = arr[k][:, :, None] == lane[None, None, :]  # [t, e, d]
        m4[k] = eq.transpose(1, 0, 2)  # [e, t, d]
        mT[k] = eq.transpose(2, 0, 1)  # [d, t, e]

    # parameter-only fusion: a_src = h @ att_src == x @ w_src
    W4 = np.asarray(W, np.float32).reshape(IN, HEADS, OUT_CH)
    w_src = np.einsum("ihc,hc->ih", W4, np.asarray(att_src, np.float32))
    w_dst = np.einsum("ihc,hc->ih", W4, np.asarray(att_dst, np.float32))
    Waug = np.ascontiguousarray(
        np.concatenate([np.asarray(W, np.float32), w_src, w_dst], axis=1)
    ).astype(np.float16)  # [IN, IN + 2*HEADS]

    xT = np.ascontiguousarray(x.T).astype(np.float16)  # [IN, N]

    meta = dict(
        N=N,
        IN=IN,
        npc=npc,
        chunks=chunks,
        Tch=Tch,
        chunk_off=chunk_off,
        TT=TT,
    )
    in_maps = []
    for k in range(n_cores):
        in_maps.append(
            {
                # halo-expanded source features, edge-slot order: [IN, TT*128]
                "xET": np.ascontiguousarray(xT[:, src_pad[k]]),
                # this core's own nodes' features: [IN, npc]
                "xTloc": np.ascontiguousarray(xT[:, k * npc : (k + 1) * npc]),
                "Waug": Waug,
                "m4": m4[k].reshape(P, TT * P),
                "mT": mT[k].reshape(P, TT * P),
                "bias": np.asarray(bias, np.float32),
            }
        )
    return meta, in_maps


# --------------------------------------------------------------------------
# device program (identical on every core)
# --------------------------------------------------------------------------
def _build_program(meta, n_cores, debug=False):
    import os

    import concourse.bacc as bacc
    import concourse.mybir as mybir
    import concourse.tile as tile


    f32 = mybir.dt.float32
    f16 = mybir.dt.float16

    N, IN = meta["N"], meta["IN"]
    npc, chunks = meta["npc"], meta["chunks"]
    Tch, chunk_off = meta["Tch"], meta["chunk_off"]
    TT = meta["TT"]
    AUG = IN + 2 * HEADS  # 264
    HS = IN + HEADS  # 260: [h | e-logit] working row
    KB = IN // P  # contraction blocks (2)

    nc = bacc.Bacc(
        "TRN2", target_bir_lowering=False, debug=debug, num_devices=n_cores
    )

    def mm(out, lhsT, rhs, **kw):
        nc.tensor.matmul(out, lhsT, rhs, **kw)

    xET_d = nc.dram_tensor("xET", [IN, TT * P], f16, kind="ExternalInput")
    xTloc_d = nc.dram_tensor("xTloc", [IN, npc], f16, kind="ExternalInput")
    Waug_d = nc.dram_tensor("Waug", [IN, AUG], f16, kind="ExternalInput")
    m4_d = nc.dram_tensor("m4", [P, TT * P], f16, kind="ExternalInput")
    mT_d = nc.dram_tensor("mT", [P, TT * P], f16, kind="ExternalInput")
    bias_d = nc.dram_tensor("bias", [IN], f32, kind="ExternalInput")
    out_d = nc.dram_tensor("out", [npc, IN], f32, kind="ExternalOutput")

    with tile.TileContext(nc) as tc:
        with tc.tile_pool(name="const", bufs=1) as cpool:
            ones_row = cpool.tile([1, P], f32)
            nc.vector.memset(ones_row[:], 1.0)

            bias_row = cpool.tile([1, IN], f32)
            nc.sync.dma_start(out=bias_row[:], in_=bias_d[None, :])
            bias_full = cpool.tile([P, HEADS, OUT_CH], f32)
            with tc.tile_pool(name="cpsum", bufs=1, space="PSUM") as cpsum:
                bias_psum = cpsum.tile([P, HEADS, OUT_CH], f32)
                nc.tensor.matmul(
                    bias_psum[:], ones_row[:], bias_row[:], start=True, stop=True
                )
                nc.vector.tensor_copy(bias_full[:], bias_psum[:])

            Waug_sb = cpool.tile([P, KB, AUG], f16)
            for k in range(KB):
                nc.sync.dma_start(
                    out=Waug_sb[:, k, :], in_=Waug_d[k * P : (k + 1) * P, :]
                )

            # ------------------------------------------------------------
            # precompute: a_dst of every local dst node, per chunk
            # ------------------------------------------------------------
            adst_all = cpool.tile([P, chunks, HEADS], f16)
            with (
                tc.tile_pool(name="locx", bufs=3) as lpool,
                tc.tile_pool(name="lpsum", bufs=2, space="PSUM") as lpsum,
            ):
                for c in range(chunks):
                    pc = min(P, npc - c * P)
                    xl = lpool.tile([P, KB, P], f16)
                    for k in range(KB):
                        nc.sync.dma_start(
                            out=xl[:, k, :pc],
                            in_=xTloc_d[
                                k * P : (k + 1) * P, c * P : c * P + pc
                            ],
                        )
                    ap = lpsum.tile([P, HEADS], f32)
                    for k in range(KB):
                        mm(
                            ap[:pc, :],
                            xl[:, k, :pc],
                            Waug_sb[:, k, IN + HEADS : IN + 2 * HEADS],
                            start=(k == 0),
                            stop=(k == KB - 1),
                        )
                    if pc < P:
                        nc.vector.memset(adst_all[:, c, :], 0.0)
                    nc.vector.tensor_copy(adst_all[:pc, c, :], ap[:pc, :])

            # ------------------------------------------------------------
            # main: per dst-chunk fused h-recompute + edge aggregation
            # ------------------------------------------------------------
            with (
                tc.tile_pool(name="xload", bufs=3) as xpool,
                tc.tile_pool(name="hrow", bufs=3) as hpool,
                tc.tile_pool(name="meta2", bufs=2) as mpool,
                tc.tile_pool(name="work", bufs=4) as wpool,
                tc.tile_pool(name="rhs", bufs=4) as rpool,
                tc.tile_pool(name="tail", bufs=2) as fpool,
                tc.tile_pool(name="hpsum", bufs=4, space="PSUM") as hpsum,
                tc.tile_pool(name="opsum", bufs=2, space="PSUM") as opsum,
                tc.tile_pool(name="apsum", bufs=2, space="PSUM") as apsum,
            ):
                for c in range(chunks):
                    Tc = int(Tch[c])
                    toff = int(chunk_off[c]) // P
                    pc = min(P, npc - c * P)

                    m4_sb = mpool.tile([P, Tc, P], f16, tag="m4")
                    nc.sync.dma_start(
                        out=m4_sb[:],
                        in_=m4_d[:, toff * P : (toff + Tc) * P].rearrange(
                            "p (t e) -> p t e", t=Tc
                        ),
                    )
                    mT_sb = mpool.tile([P, Tc, P], f16, tag="mT")
                    nc.sync.dma_start(
                        out=mT_sb[:],
                        in_=mT_d[:, toff * P : (toff + Tc) * P].rearrange(
                            "p (t e) -> p t e", t=Tc
                        ),
                    )

                    # hoisted per-edge a_dst for the whole chunk: Tc small
                    # matmuls into one PSUM bank, then one fp16 copy to SBUF
                    aep_ps = apsum.tile([P, Tc, HEADS], f32)
                    for t in range(Tc):
                        mm(
                            aep_ps[:, t, :],
                            mT_sb[:, t, :],
                            adst_all[:, c, :],
                            start=True,
                            stop=True,
                        )
                    aep_sb = mpool.tile([P, Tc, HEADS], f16, tag="aep")
                    nc.vector.tensor_copy(aep_sb[:], aep_ps[:])

                    out_ps = opsum.tile([P, 4, 65], f32)
                    for gg in range(0, Tc, 8):
                        ngg = min(8, Tc - gg)
                        xe = xpool.tile([P, KB, 8 * P], f16, tag="xe")
                        for k in range(KB):
                            nc.sync.dma_start(
                                out=xe[:, k, : ngg * P],
                                in_=xET_d[
                                    k * P : (k + 1) * P,
                                    (toff + gg) * P : (toff + gg + ngg) * P,
                                ],
                            )
                        for t in range(gg, gg + ngg, 4):
                            nb = min(4, Tc - t)
                            q0 = t - gg
                            # tiles 0..nbp-1: PSUM-direct (DVE multiplies
                            # straight out of PSUM, no fp16 cast).  tiles
                            # nbp..nb-1: cast to SBUF on the scalar engine,
                            # multiply on the (otherwise idle) gpsimd engine.
                            nbp = min(2, nb)
                            hps = []
                            hs4 = hpool.tile([P, 4, HS], f16, tag="hs")
                            for i in range(nb):
                                hp = hpsum.tile([P, HS], f32)
                                for k in range(KB):
                                    mm(
                                        hp[:],
                                        xe[:, k, (q0 + i) * P : (q0 + i + 1) * P],
                                        Waug_sb[:, k, 0:HS],
                                        start=(k == 0),
                                        stop=(k == KB - 1),
                                    )
                                hps.append(hp)
                                if i >= nbp:
                                    nc.scalar.activation(
                                        hs4[:, i, :],
                                        hp[:],
                                        mybir.ActivationFunctionType.Copy,
                                    )
                            # e = a_src + a_dst, per edge
                            e04 = wpool.tile([P, 4, HEADS], f32, tag="e0")
                            for i in range(nbp):
                                nc.vector.tensor_add(
                                    e04[:, i, :],
                                    hps[i][:, IN : IN + HEADS],
                                    aep_sb[:, t + i, :],
                                )
                            if nb > nbp:
                                nc.vector.tensor_add(
                                    e04[:, nbp:nb, :],
                                    hs4[:, nbp:nb, IN : IN + HEADS],
                                    aep_sb[:, t + nbp : t + nb, :],
                                )
                            # exp(lrelu(e)) == exp(0.2*(e + 4*relu(e)))
                            epos4 = wpool.tile([P, 4, HEADS], f32, tag="ep")
                            nc.vector.tensor_scalar(
                                epos4[:, :nb, :],
                                e04[:, :nb, :],
                                0.0,
                                4.0,
                                mybir.AluOpType.max,
                                mybir.AluOpType.mult,
                            )
                            el4 = wpool.tile([P, 4, HEADS], f32, tag="el")
                            nc.vector.tensor_add(
                                el4[:, :nb, :], epos4[:, :nb, :], e04[:, :nb, :]
                            )
                            brhs = rpool.tile([P, 4, 4, 65], f16, tag="grhs")
                            nc.scalar.activation(
                                brhs[:, :nb, :, 64],
                                el4[:, :nb, :],
                                mybir.ActivationFunctionType.Exp,
                                scale=NEG_SLOPE,
                            )
                            for i in range(nbp):
                                nc.vector.tensor_tensor(
                                    out=brhs[:, i, :, 0:64],
                                    in0=hps[i][:, 0:IN].rearrange(
                                        "p (h c) -> p h c", h=HEADS
                                    ),
                                    in1=brhs[:, i, :, 64:65].to_broadcast(
                                        [P, HEADS, OUT_CH]
                                    ),
                                    op=mybir.AluOpType.mult,
                                )
                            if nb > nbp:
                                nc.gpsimd.tensor_tensor(
                                    out=brhs[:, nbp:nb, :, 0:64],
                                    in0=hs4[:, nbp:nb, 0:IN].rearrange(
                                        "p q (h c) -> p q h c", h=HEADS
                                    ),
                                    in1=brhs[:, nbp:nb, :, 64:65].to_broadcast(
                                        [P, nb - nbp, HEADS, OUT_CH]
                                    ),
                                    op=mybir.AluOpType.mult,
                                )
                            for i in range(nb):
                                mm(
                                    out_ps[:],
                                    m4_sb[:, t + i, :],
                                    brhs[:, i],
                                    start=(t + i == 0),
                                    stop=(t + i == Tc - 1),
                                )
                    # chunk tail: softmax division, bias, relu, L2 norm
                    dn = fpool.tile([P, HEADS], f32, tag="dn")
                    nc.vector.tensor_scalar_max(dn[:], out_ps[:, :, 64], 1e-30)
                    rdn = fpool.tile([P, HEADS], f32, tag="rdn")
                    nc.vector.reciprocal(rdn[:], dn[:])
                    o1 = fpool.tile([P, HEADS, OUT_CH], f32, tag="o1")
                    nc.vector.tensor_tensor(
                        out=o1[:],
                        in0=out_ps[:, :, 0:64],
                        in1=rdn[:, :, None].to_broadcast([P, HEADS, OUT_CH]),
                        op=mybir.AluOpType.mult,
                    )
                    nc.vector.tensor_add(o1[:], o1[:], bias_full[:])
                    o2 = fpool.tile([P, HEADS, OUT_CH], f32, tag="o2")
                    nc.scalar.activation(
                        o2[:], o1[:], mybir.ActivationFunctionType.Relu
                    )
                    sq = fpool.tile([P, HEADS, OUT_CH], f32, tag="sq")
                    nc.gpsimd.tensor_mul(sq[:], o2[:], o2[:])
                    s = fpool.tile([P, 1], f32, tag="s")
                    nc.vector.tensor_reduce(
                        s[:],
                        sq[:],
                        axis=mybir.AxisListType.XY,
                        op=mybir.AluOpType.add,
                    )
                    r = fpool.tile([P, 1], f32, tag="r")
                    nc.scalar.sqrt(r[:], s[:])
                    nc.vector.tensor_scalar_max(r[:], r[:], 1e-12)
                    rr = fpool.tile([P, 1], f32, tag="rr")
                    nc.vector.reciprocal(rr[:], r[:])
                    o3 = fpool.tile([P, HEADS, OUT_CH], f32, tag="o3")
                    nc.vector.tensor_scalar_mul(o3[:], o2[:], rr[:])
                    nc.sync.dma_start(
                        out=out_d[c * P : c * P + pc, :], in_=o3[:pc]
                    )

    nc.compile()
    return nc


# --------------------------------------------------------------------------
# entry point: full inputs in, full output out
# --------------------------------------------------------------------------
def kernel(x, edge_index, W, att_src, att_dst, bias):
    from concourse.bass_utils import run_bass_kernel_spmd

    n_cores = 8
    meta, in_maps = _preprocess(x, edge_index, W, att_src, att_dst, bias, n_cores)
    nc = _build_program(meta, n_cores)
    res = run_bass_kernel_spmd(nc, in_maps, list(range(n_cores)))
    out = np.concatenate([res.results[k]["out"] for k in range(n_cores)], axis=0)
    return out.astype(np.float32)


# revision 20
# speedup vs baseline: 1.3924x; 1.3924x over previous
"""GAT (graph attention) kernel for 8 Trainium2 NeuronCores.

Strategy (1D dst-partitioning + host-side halo expansion):
  * Core k owns dst nodes [k*npc, (k+1)*npc).  Host appends self-loops and
    buckets edges by (dst core, dst chunk of 128), padding each chunk's
    bucket to a multiple of 128 edges with uniform tile counts across cores,
    so ONE SPMD program serves all 8 cores.
  * Instead of an on-device gather of source-node features (SWDGE
    descriptor generation is ~8ns/edge, a 1.8ms serial floor on GpSimd),
    the HOST expands the halo: xE[slot] = x[src[slot]] in edge-slot order.
    Each core streams its xE slabs contiguously and recomputes
    h_e = xE_e @ [W | w_src] on the tensor engine per edge slot (PE has
    headroom; 4.4x redundant h-flops beat the descriptor-gen wall).
  * Host also ships the per-tile one-hot scatter matrices (fp16 0/1):
    mask4[e,d] (edge->dst) and its transpose maskT[d,e] — DMA is cheaper
    than building them with iota/is_equal on the DVE.
  * Device, per dst chunk c (128 dst nodes), per 128-edge tile:
      hp   = xE_tile @ [W | w_src]          (PSUM, fp16 matmul)
      aep  = maskT @ a_dst_chunk            (per-edge dst logit)
      e    = hp[:,256:260] + aep; expE = exp(leaky_relu(e))
      brhs = [h * expE | expE]              (fp16)
      out_ps[d] += mask4^T @ brhs           (PSUM chain over the chunk:
                                             numerator + denominator)
    a_dst_chunk comes from a tiny precompute pass over the core's own
    x rows (4-column matmul).  Softmax division, bias, relu, L2-normalize
    on the chunk tail.  exp() skips the segment-max shift: logits are O(10)
    so exp stays in range, and softmax is shift-invariant.
  * PSUM->SBUF fp16 casts run on the (otherwise idle) GpSimd engine to
    keep the DVE off the critical path.
"""

import sys

sys.path.insert(0, "/opt/trn_rl_repo")

import numpy as np

HEADS = 4
OUT_CH = 64
NEG_SLOPE = 0.2
P = 128


# --------------------------------------------------------------------------
# host-side preprocessing (sharding + layout only, plus parameter fusion)
# --------------------------------------------------------------------------
def _preprocess(x, edge_index, W, att_src, att_dst, bias, n_cores):
    x = np.asarray(x, np.float32)
    N, IN = x.shape
    assert N % n_cores == 0
    npc = N // n_cores
    chunks = (npc + P - 1) // P

    src = np.concatenate(
        [np.asarray(edge_index[0], np.int64), np.arange(N, dtype=np.int64)]
    )
    dst = np.concatenate(
        [np.asarray(edge_index[1], np.int64), np.arange(N, dtype=np.int64)]
    )

    core = dst // npc
    rem = dst - core * npc
    chunk = rem // P
    dstl = (rem - chunk * P).astype(np.float32)

    # per-core edge buckets by dst chunk
    per_core = []
    for k in range(n_cores):
        sel = np.nonzero(core == k)[0]
        key = chunk[sel]
        order = np.argsort(key, kind="stable")
        counts = np.bincount(key, minlength=chunks)
        starts = np.zeros(chunks + 1, np.int64)
        np.cumsum(counts, out=starts[1:])
        per_core.append((src[sel][order], dstl[sel][order], counts, starts))

    all_counts = np.stack([pc[2] for pc in per_core])  # [cores, chunks]
    Tch = np.maximum(1, -(-all_counts.max(axis=0) // P))  # [chunks]
    slots_per_chunk = P * Tch
    total_slots = int(slots_per_chunk.sum())
    TT = int(total_slots // P)

    chunk_off = np.zeros(chunks + 1, np.int64)
    np.cumsum(slots_per_chunk, out=chunk_off[1:])

    src_pad = np.zeros((n_cores, total_slots), np.int64)
    dstl_pad = np.full((n_cores, total_slots), -1.0, np.float32)
    for k in range(n_cores):
        src_s, dstl_s, counts, starts = per_core[k]
        for c in range(chunks):
            off = int(chunk_off[c])
            s0, s1 = int(starts[c]), int(starts[c + 1])
            n = s1 - s0
            src_pad[k, off : off + n] = src_s[s0:s1]
            dstl_pad[k, off : off + n] = dstl_s[s0:s1]

    # host-built one-hot scatter matrices (fp16 0/1):
    #   mask4[e, t, d] = 1 if edge slot (t*128+e) targets local dst d
    #   maskT[d, t, e] = transpose of the same
    lane = np.arange(P, dtype=np.float32)
    arr = dstl_pad.reshape(n_cores, TT, P)  # [k, t, e]
    m4 = np.empty((n_cores, P, TT, P), np.float16)
    mT = np.empty((n_cores, P, TT, P), np.float16)
    for k in range(n_cores):
        eq = arr[k][:, :, None] == lane[None, None, :]  # [t, e, d]
        m4[k] = eq.transpose(1, 0, 2)  # [e, t, d]
        mT[k] = eq.transpose(2, 0, 1)  # [d, t, e]

    # parameter-only fusion: a_src = h @ att_src == x @ w_src
    W4 = np.asarray(W, np.float32).reshape(IN, HEADS, OUT_CH)
    w_src = np.einsum("ihc,hc->ih", W4, np.asarray(att_src, np.float32))
    w_dst = np.einsum("ihc,hc->ih", W4, np.asarray(att_dst, np.float32))
    Waug = np.ascontiguousarray(
        np.concatenate([np.asarray(W, np.float32), w_src, w_dst], axis=1)
    ).astype(np.float16)  # [IN, IN + 2*HEADS]

    xT = np.ascontiguousarray(x.T).astype(np.float16)  # [IN, N]

    meta = dict(
        N=N,
        IN=IN,
        npc=npc,
        chunks=chunks,
        Tch=Tch,
        chunk_off=chunk_off,
        TT=TT,
    )
    in_maps = []
    for k in range(n_cores):
        in_maps.append(
            {
                # halo-expanded source features, edge-slot order: [IN, TT*128]
                "xET": np.ascontiguousarray(xT[:, src_pad[k]]),
                # this core's own nodes' features: [IN, npc]
                "xTloc": np.ascontiguousarray(xT[:, k * npc : (k + 1) * npc]),
                "Waug": Waug,
                "m4": m4[k].reshape(P, TT * P),
                "mT": mT[k].reshape(P, TT * P),
                "bias": np.asarray(bias, np.float32),
            }
        )
    return meta, in_maps


# --------------------------------------------------------------------------
# device program (identical on every core)
# --------------------------------------------------------------------------
def _build_program(meta, n_cores, debug=False):
    import os

    import concourse.bacc as bacc
    import concourse.mybir as mybir
    import concourse.tile as tile


    f32 = mybir.dt.float32
    f16 = mybir.dt.float16

    N, IN = meta["N"], meta["IN"]
    npc, chunks = meta["npc"], meta["chunks"]
    Tch, chunk_off = meta["Tch"], meta["chunk_off"]
    TT = meta["TT"]
    AUG = IN + 2 * HEADS  # 264
    HS = IN + HEADS  # 260: [h | e-logit] working row
    KB = IN // P  # contraction blocks (2)

    nc = bacc.Bacc(
        "TRN2", target_bir_lowering=False, debug=debug, num_devices=n_cores
    )

    def mm(out, lhsT, rhs, **kw):
        nc.tensor.matmul(out, lhsT, rhs, **kw)

    xET_d = nc.dram_tensor("xET", [IN, TT * P], f16, kind="ExternalInput")
    xTloc_d = nc.dram_tensor("xTloc", [IN, npc], f16, kind="ExternalInput")
    Waug_d = nc.dram_tensor("Waug", [IN, AUG], f16, kind="ExternalInput")
    m4_d = nc.dram_tensor("m4", [P, TT * P], f16, kind="ExternalInput")
    mT_d = nc.dram_tensor("mT", [P, TT * P], f16, kind="ExternalInput")
    bias_d = nc.dram_tensor("bias", [IN], f32, kind="ExternalInput")
    out_d = nc.dram_tensor("out", [npc, IN], f32, kind="ExternalOutput")

    with tile.TileContext(nc) as tc:
        with tc.tile_pool(name="const", bufs=1) as cpool:
            ones_row = cpool.tile([1, P], f32)
            nc.vector.memset(ones_row[:], 1.0)

            bias_row = cpool.tile([1, IN], f32)
            nc.sync.dma_start(out=bias_row[:], in_=bias_d[None, :])
            bias_full = cpool.tile([P, HEADS, OUT_CH], f32)
            with tc.tile_pool(name="cpsum", bufs=1, space="PSUM") as cpsum:
                bias_psum = cpsum.tile([P, HEADS, OUT_CH], f32)
                nc.tensor.matmul(
                    bias_psum[:], ones_row[:], bias_row[:], start=True, stop=True
                )
                nc.vector.tensor_copy(bias_full[:], bias_psum[:])

            Waug_sb = cpool.tile([P, KB, AUG], f16)
            for k in range(KB):
                nc.sync.dma_start(
                    out=Waug_sb[:, k, :], in_=Waug_d[k * P : (k + 1) * P, :]
                )

            # ------------------------------------------------------------
            # precompute: a_dst of every local dst node, per chunk
            # ------------------------------------------------------------
            adst_all = cpool.tile([P, chunks, HEADS], f16)
            with (
                tc.tile_pool(name="locx", bufs=3) as lpool,
                tc.tile_pool(name="lpsum", bufs=2, space="PSUM") as lpsum,
            ):
                for c in range(chunks):
                    pc = min(P, npc - c * P)
                    xl = lpool.tile([P, KB, P], f16)
                    for k in range(KB):
                        nc.sync.dma_start(
                            out=xl[:, k, :pc],
                            in_=xTloc_d[
                                k * P : (k + 1) * P, c * P : c * P + pc
                            ],
                        )
                    ap = lpsum.tile([P, HEADS], f32)
                    for k in range(KB):
                        mm(
                            ap[:pc, :],
                            xl[:, k, :pc],
                            Waug_sb[:, k, IN + HEADS : IN + 2 * HEADS],
                            start=(k == 0),
                            stop=(k == KB - 1),
                        )
                    if pc < P:
                        nc.vector.memset(adst_all[:, c, :], 0.0)
                    nc.vector.tensor_copy(adst_all[:pc, c, :], ap[:pc, :])

            # ------------------------------------------------------------
            # main: per dst-chunk fused h-recompute + edge aggregation
            # ------------------------------------------------------------
            with (
                tc.tile_pool(name="xload", bufs=3) as xpool,
                tc.tile_pool(name="hrow", bufs=3) as hpool,
                tc.tile_pool(name="meta2", bufs=2) as mpool,
                tc.tile_pool(name="work", bufs=4) as wpool,
                tc.tile_pool(name="rhs", bufs=4) as rpool,
                tc.tile_pool(name="tail", bufs=2) as fpool,
                tc.tile_pool(name="hpsum", bufs=4, space="PSUM") as hpsum,
                tc.tile_pool(name="opsum", bufs=2, space="PSUM") as opsum,
                tc.tile_pool(name="apsum", bufs=2, space="PSUM") as apsum,
            ):
                for c in range(chunks):
                    Tc = int(Tch[c])
                    toff = int(chunk_off[c]) // P
                    pc = min(P, npc - c * P)

                    m4_sb = mpool.tile([P, Tc, P], f16, tag="m4")
                    nc.sync.dma_start(
                        out=m4_sb[:],
                        in_=m4_d[:, toff * P : (toff + Tc) * P].rearrange(
                            "p (t e) -> p t e", t=Tc
                        ),
                    )
                    mT_sb = mpool.tile([P, Tc, P], f16, tag="mT")
                    nc.sync.dma_start(
                        out=mT_sb[:],
                        in_=mT_d[:, toff * P : (toff + Tc) * P].rearrange(
                            "p (t e) -> p t e", t=Tc
                        ),
                    )

                    out_ps = opsum.tile([P, 4, 65], f32)
                    for gg in range(0, Tc, 8):
                        ngg = min(8, Tc - gg)
                        xe = xpool.tile([P, KB, 8 * P], f16, tag="xe")
                        for k in range(KB):
                            nc.sync.dma_start(
                                out=xe[:, k, : ngg * P],
                                in_=xET_d[
                                    k * P : (k + 1) * P,
                                    (toff + gg) * P : (toff + gg + ngg) * P,
                                ],
                            )
                        for t in range(gg, gg + ngg, 4):
                            nb = min(4, Tc - t)
                            q0 = t - gg
                            hs4 = hpool.tile([P, 4, HS], f16, tag="hs")
                            for i in range(nb):
                                hp = hpsum.tile([P, HS], f32)
                                for k in range(KB):
                                    mm(
                                        hp[:],
                                        xe[:, k, (q0 + i) * P : (q0 + i + 1) * P],
                                        Waug_sb[:, k, 0:HS],
                                        start=(k == 0),
                                        stop=(k == KB - 1),
                                    )
                                if i % 2 == 1:
                                    nc.scalar.activation(
                                        hs4[:, i, :],
                                        hp[:],
                                        mybir.ActivationFunctionType.Copy,
                                    )
                                else:
                                    nc.vector.tensor_copy(hs4[:, i, :], hp[:])
                            aep4 = apsum.tile([P, 4, HEADS], f32)
                            for i in range(nb):
                                mm(
                                    aep4[:, i, :],
                                    mT_sb[:, t + i, :],
                                    adst_all[:, c, :],
                                    start=True,
                                    stop=True,
                                )
                            e04 = wpool.tile([P, 4, HEADS], f32, tag="e0")
                            nc.vector.tensor_add(
                                e04[:, :nb, :],
                                hs4[:, :nb, IN : IN + HEADS],
                                aep4[:, :nb, :],
                            )
                            # exp(lrelu(e)) == exp(0.2*(e + 4*relu(e)))
                            epos4 = wpool.tile([P, 4, HEADS], f32, tag="ep")
                            nc.scalar.activation(
                                epos4[:, :nb, :],
                                e04[:, :nb, :],
                                mybir.ActivationFunctionType.Relu,
                            )
                            el4 = wpool.tile([P, 4, HEADS], f32, tag="el")
                            nc.vector.scalar_tensor_tensor(
                                out=el4[:, :nb, :],
                                in0=epos4[:, :nb, :],
                                scalar=4.0,
                                in1=e04[:, :nb, :],
                                op0=mybir.AluOpType.mult,
                                op1=mybir.AluOpType.add,
                            )
                            brhs = rpool.tile([P, 4, 4, 65], f16, tag="grhs")
                            nc.scalar.activation(
                                brhs[:, :nb, :, 64],
                                el4[:, :nb, :],
                                mybir.ActivationFunctionType.Exp,
                                scale=NEG_SLOPE,
                            )
                            nbd = min(1, nb)
                            nc.vector.tensor_tensor(
                                out=brhs[:, :nbd, :, 0:64],
                                in0=hs4[:, :nbd, 0:IN].rearrange(
                                    "p q (h c) -> p q h c", h=HEADS
                                ),
                                in1=brhs[:, :nbd, :, 64:65].to_broadcast(
                                    [P, nbd, HEADS, OUT_CH]
                                ),
                                op=mybir.AluOpType.mult,
                            )
                            if nb > nbd:
                                nc.gpsimd.tensor_tensor(
                                    out=brhs[:, nbd:nb, :, 0:64],
                                    in0=hs4[:, nbd:nb, 0:IN].rearrange(
                                        "p q (h c) -> p q h c", h=HEADS
                                    ),
                                    in1=brhs[:, nbd:nb, :, 64:65].to_broadcast(
                                        [P, nb - nbd, HEADS, OUT_CH]
                                    ),
                                    op=mybir.AluOpType.mult,
                                )
                            for i in range(nb):
                                mm(
                                    out_ps[:],
                                    m4_sb[:, t + i, :],
                                    brhs[:, i],
                                    start=(t + i == 0),
                                    stop=(t + i == Tc - 1),
                                )
                    # chunk tail: softmax division, bias, relu, L2 norm
                    dn = fpool.tile([P, HEADS], f32, tag="dn")
                    nc.vector.tensor_scalar_max(dn[:], out_ps[:, :, 64], 1e-30)
                    rdn = fpool.tile([P, HEADS], f32, tag="rdn")
                    nc.vector.reciprocal(rdn[:], dn[:])
                    o1 = fpool.tile([P, HEADS, OUT_CH], f32, tag="o1")
                    nc.vector.tensor_tensor(
                        out=o1[:],
                        in0=out_ps[:, :, 0:64],
                        in1=rdn[:, :, None].to_broadcast([P, HEADS, OUT_CH]),
                        op=mybir.AluOpType.mult,
                    )
                    nc.vector.tensor_add(o1[:], o1[:], bias_full[:])
                    o2 = fpool.tile([P, HEADS, OUT_CH], f32, tag="o2")
                    nc.scalar.activation(
                        o2[:], o1[:], mybir.ActivationFunctionType.Relu
                    )
                    sq = fpool.tile([P, HEADS, OUT_CH], f32, tag="sq")
                    nc.gpsimd.tensor_mul(sq[:], o2[:], o2[:])
                    s = fpool.tile([P, 1], f32, tag="s")
                    nc.vector.tensor_reduce(
                        s[:],
                        sq[:],
                        axis=mybir.AxisListType.XY,
                        op=mybir.AluOpType.add,
                    )
                    r = fpool.tile([P, 1], f32, tag="r")
                    nc.scalar.sqrt(r[:], s[:])
                    nc.vector.tensor_scalar_max(r[:], r[:], 1e-12)
                    rr = fpool.tile([P, 1], f32, tag="rr")
                    nc.vector.reciprocal(rr[:], r[:])
                    o3 = fpool.tile([P, HEADS, OUT_CH], f32, tag="o3")
                    nc.vector.tensor_scalar_mul(o3[:], o2[:], rr[:])
                    nc.sync.dma_start(
                        out=out_d[c * P : c * P + pc, :], in_=o3[:pc]
                    )

    nc.compile()
    return nc


# --------------------------------------------------------------------------
# entry point: full inputs in, full output out
# --------------------------------------------------------------------------
def kernel(x, edge_index, W, att_src, att_dst, bias):
    from concourse.bass_utils import run_bass_kernel_spmd

    n_cores = 8
    meta, in_maps = _preprocess(x, edge_index, W, att_src, att_dst, bias, n_cores)
    nc = _build_program(meta, n_cores)
    res = run_bass_kernel_spmd(nc, in_maps, list(range(n_cores)))
    out = np.concatenate([res.results[k]["out"] for k in range(n_cores)], axis=0)
    return out.astype(np.float32)


# revision 23
# speedup vs baseline: 1.4529x; 1.0435x over previous
"""GAT (graph attention) kernel for 8 Trainium2 NeuronCores.

Strategy (1D dst-partitioning + host-side halo expansion):
  * Core k owns dst nodes [k*npc, (k+1)*npc).  Host appends self-loops and
    buckets edges by (dst core, dst chunk of 128), padding each chunk's
    bucket to a multiple of 128 edges with uniform tile counts across cores,
    so ONE SPMD program serves all 8 cores.
  * Instead of an on-device gather of source-node features (SWDGE
    descriptor generation is ~8ns/edge, a 1.8ms serial floor on GpSimd),
    the HOST expands the halo: xE[slot] = x[src[slot]] in edge-slot order.
    Each core streams its xE slabs contiguously and recomputes
    h_e = xE_e @ [W | w_src] on the tensor engine per edge slot (PE has
    headroom; 4.4x redundant h-flops beat the descriptor-gen wall).
  * Host also ships the per-tile one-hot scatter matrices (fp16 0/1):
    mask4[e,d] (edge->dst) and its transpose maskT[d,e] — DMA is cheaper
    than building them with iota/is_equal on the DVE.
  * Device, per dst chunk c (128 dst nodes), per 128-edge tile:
      hp   = xE_tile @ [W | w_src]          (PSUM, fp16 matmul)
      aep  = maskT @ a_dst_chunk            (per-edge dst logit)
      e    = hp[:,256:260] + aep; expE = exp(leaky_relu(e))
      brhs = [h * expE | expE]              (fp16)
      out_ps[d] += mask4^T @ brhs           (PSUM chain over the chunk:
                                             numerator + denominator)
    a_dst_chunk comes from a tiny precompute pass over the core's own
    x rows (4-column matmul).  Softmax division, bias, relu, L2-normalize
    on the chunk tail.  exp() skips the segment-max shift: logits are O(10)
    so exp stays in range, and softmax is shift-invariant.
  * PSUM->SBUF fp16 casts run on the (otherwise idle) GpSimd engine to
    keep the DVE off the critical path.
"""

import sys

sys.path.insert(0, "/opt/trn_rl_repo")

import numpy as np

HEADS = 4
OUT_CH = 64
NEG_SLOPE = 0.2
P = 128


# --------------------------------------------------------------------------
# host-side preprocessing (sharding + layout only, plus parameter fusion)
# --------------------------------------------------------------------------
def _preprocess(x, edge_index, W, att_src, att_dst, bias, n_cores):
    x = np.asarray(x, np.float32)
    N, IN = x.shape
    assert N % n_cores == 0
    npc = N // n_cores
    chunks = (npc + P - 1) // P

    src = np.concatenate(
        [np.asarray(edge_index[0], np.int64), np.arange(N, dtype=np.int64)]
    )
    dst = np.concatenate(
        [np.asarray(edge_index[1], np.int64), np.arange(N, dtype=np.int64)]
    )

    core = dst // npc
    rem = dst - core * npc
    chunk = rem // P
    dstl = (rem - chunk * P).astype(np.float32)

    # per-core edge buckets by dst chunk
    per_core = []
    for k in range(n_cores):
        sel = np.nonzero(core == k)[0]
        key = chunk[sel]
        order = np.argsort(key, kind="stable")
        counts = np.bincount(key, minlength=chunks)
        starts = np.zeros(chunks + 1, np.int64)
        np.cumsum(counts, out=starts[1:])
        per_core.append((src[sel][order], dstl[sel][order], counts, starts))

    all_counts = np.stack([pc[2] for pc in per_core])  # [cores, chunks]
    Tch = np.maximum(1, -(-all_counts.max(axis=0) // P))  # [chunks]
    slots_per_chunk = P * Tch
    total_slots = int(slots_per_chunk.sum())
    TT = int(total_slots // P)

    chunk_off = np.zeros(chunks + 1, np.int64)
    np.cumsum(slots_per_chunk, out=chunk_off[1:])

    src_pad = np.zeros((n_cores, total_slots), np.int64)
    dstl_pad = np.full((n_cores, total_slots), -1.0, np.float32)
    for k in range(n_cores):
        src_s, dstl_s, counts, starts = per_core[k]
        for c in range(chunks):
            off = int(chunk_off[c])
            s0, s1 = int(starts[c]), int(starts[c + 1])
            n = s1 - s0
            src_pad[k, off : off + n] = src_s[s0:s1]
            dstl_pad[k, off : off + n] = dstl_s[s0:s1]

    # host-built one-hot scatter matrices (fp16 0/1):
    #   mask4[e, t, d] = 1 if edge slot (t*128+e) targets local dst d
    #   maskT[d, t, e] = transpose of the same
    lane = np.arange(P, dtype=np.float32)
    arr = dstl_pad.reshape(n_cores, TT, P)  # [k, t, e]
    m4 = np.empty((n_cores, P, TT, P), np.float16)
    mT = np.empty((n_cores, P, TT, P), np.float16)
    for k in range(n_cores):
        eq = arr[k][:, :, None] == lane[None, None, :]  # [t, e, d]
        m4[k] = eq.transpose(1, 0, 2)  # [e, t, d]
        mT[k] = eq.transpose(2, 0, 1)  # [d, t, e]

    # parameter-only fusion: a_src = h @ att_src == x @ w_src
    W4 = np.asarray(W, np.float32).reshape(IN, HEADS, OUT_CH)
    w_src = np.einsum("ihc,hc->ih", W4, np.asarray(att_src, np.float32))
    w_dst = np.einsum("ihc,hc->ih", W4, np.asarray(att_dst, np.float32))
    Waug = np.ascontiguousarray(
        np.concatenate([np.asarray(W, np.float32), w_src, w_dst], axis=1)
    ).astype(np.float16)  # [IN, IN + 2*HEADS]

    xT = np.ascontiguousarray(x.T).astype(np.float16)  # [IN, N]

    meta = dict(
        N=N,
        IN=IN,
        npc=npc,
        chunks=chunks,
        Tch=Tch,
        chunk_off=chunk_off,
        TT=TT,
    )
    in_maps = []
    for k in range(n_cores):
        in_maps.append(
            {
                # halo-expanded source features, edge-slot order: [IN, TT*128]
                "xET": np.ascontiguousarray(xT[:, src_pad[k]]),
                # this core's own nodes' features: [IN, npc]
                "xTloc": np.ascontiguousarray(xT[:, k * npc : (k + 1) * npc]),
                "Waug": Waug,
                "m4": m4[k].reshape(P, TT * P),
                "mT": mT[k].reshape(P, TT * P),
                "bias": np.asarray(bias, np.float32),
            }
        )
    return meta, in_maps


# --------------------------------------------------------------------------
# device program (identical on every core)
# --------------------------------------------------------------------------
def _build_program(meta, n_cores, debug=False):
    import os

    import concourse.bacc as bacc
    import concourse.mybir as mybir
    import concourse.tile as tile


    f32 = mybir.dt.float32
    f16 = mybir.dt.float16

    N, IN = meta["N"], meta["IN"]
    npc, chunks = meta["npc"], meta["chunks"]
    Tch, chunk_off = meta["Tch"], meta["chunk_off"]
    TT = meta["TT"]
    AUG = IN + 2 * HEADS  # 264
    HS = IN + HEADS  # 260: [h | e-logit] working row
    KB = IN // P  # contraction blocks (2)

    nc = bacc.Bacc(
        "TRN2", target_bir_lowering=False, debug=debug, num_devices=n_cores
    )

    def mm(out, lhsT, rhs, **kw):
        nc.tensor.matmul(out, lhsT, rhs, **kw)

    xET_d = nc.dram_tensor("xET", [IN, TT * P], f16, kind="ExternalInput")
    xTloc_d = nc.dram_tensor("xTloc", [IN, npc], f16, kind="ExternalInput")
    Waug_d = nc.dram_tensor("Waug", [IN, AUG], f16, kind="ExternalInput")
    m4_d = nc.dram_tensor("m4", [P, TT * P], f16, kind="ExternalInput")
    mT_d = nc.dram_tensor("mT", [P, TT * P], f16, kind="ExternalInput")
    bias_d = nc.dram_tensor("bias", [IN], f32, kind="ExternalInput")
    out_d = nc.dram_tensor("out", [npc, IN], f32, kind="ExternalOutput")

    with tile.TileContext(nc) as tc:
        with tc.tile_pool(name="const", bufs=1) as cpool:
            ones_row = cpool.tile([1, P], f32)
            nc.vector.memset(ones_row[:], 1.0)

            bias_row = cpool.tile([1, IN], f32)
            nc.sync.dma_start(out=bias_row[:], in_=bias_d[None, :])
            bias_full = cpool.tile([P, HEADS, OUT_CH], f32)
            with tc.tile_pool(name="cpsum", bufs=1, space="PSUM") as cpsum:
                bias_psum = cpsum.tile([P, HEADS, OUT_CH], f32)
                nc.tensor.matmul(
                    bias_psum[:], ones_row[:], bias_row[:], start=True, stop=True
                )
                nc.vector.tensor_copy(bias_full[:], bias_psum[:])

            Waug_sb = cpool.tile([P, KB, AUG], f16)
            for k in range(KB):
                nc.sync.dma_start(
                    out=Waug_sb[:, k, :], in_=Waug_d[k * P : (k + 1) * P, :]
                )

            # ------------------------------------------------------------
            # precompute: a_dst of every local dst node, per chunk
            # ------------------------------------------------------------
            adst_all = cpool.tile([P, chunks, HEADS], f16)
            with (
                tc.tile_pool(name="locx", bufs=3) as lpool,
                tc.tile_pool(name="lpsum", bufs=2, space="PSUM") as lpsum,
            ):
                for c in range(chunks):
                    pc = min(P, npc - c * P)
                    xl = lpool.tile([P, KB, P], f16)
                    for k in range(KB):
                        nc.sync.dma_start(
                            out=xl[:, k, :pc],
                            in_=xTloc_d[
                                k * P : (k + 1) * P, c * P : c * P + pc
                            ],
                        )
                    ap = lpsum.tile([P, HEADS], f32)
                    for k in range(KB):
                        mm(
                            ap[:pc, :],
                            xl[:, k, :pc],
                            Waug_sb[:, k, IN + HEADS : IN + 2 * HEADS],
                            start=(k == 0),
                            stop=(k == KB - 1),
                        )
                    if pc < P:
                        nc.vector.memset(adst_all[:, c, :], 0.0)
                    nc.vector.tensor_copy(adst_all[:pc, c, :], ap[:pc, :])

            # ------------------------------------------------------------
            # main: per dst-chunk fused h-recompute + edge aggregation
            # ------------------------------------------------------------
            with (
                tc.tile_pool(name="xload", bufs=3) as xpool,
                tc.tile_pool(name="hrow", bufs=3) as hpool,
                tc.tile_pool(name="meta2", bufs=2) as mpool,
                tc.tile_pool(name="work", bufs=4) as wpool,
                tc.tile_pool(name="rhs", bufs=4) as rpool,
                tc.tile_pool(name="tail", bufs=2) as fpool,
                tc.tile_pool(name="hpsum", bufs=2, space="PSUM") as hpsum,
                tc.tile_pool(name="opsum", bufs=2, space="PSUM") as opsum,
                tc.tile_pool(name="apsum", bufs=2, space="PSUM") as apsum,
            ):
                for c in range(chunks):
                    Tc = int(Tch[c])
                    toff = int(chunk_off[c]) // P
                    pc = min(P, npc - c * P)

                    m4_sb = mpool.tile([P, Tc, P], f16, tag="m4")
                    nc.sync.dma_start(
                        out=m4_sb[:],
                        in_=m4_d[:, toff * P : (toff + Tc) * P].rearrange(
                            "p (t e) -> p t e", t=Tc
                        ),
                    )
                    mT_sb = mpool.tile([P, Tc, P], f16, tag="mT")
                    nc.sync.dma_start(
                        out=mT_sb[:],
                        in_=mT_d[:, toff * P : (toff + Tc) * P].rearrange(
                            "p (t e) -> p t e", t=Tc
                        ),
                    )

                    out_ps = opsum.tile([P, 4, 65], f32)
                    for gg in range(0, Tc, 8):
                        ngg = min(8, Tc - gg)
                        xe = xpool.tile([P, KB, 8 * P], f16, tag="xe")
                        for k in range(KB):
                            nc.sync.dma_start(
                                out=xe[:, k, : ngg * P],
                                in_=xET_d[
                                    k * P : (k + 1) * P,
                                    (toff + gg) * P : (toff + gg + ngg) * P,
                                ],
                            )
                        for t in range(gg, gg + ngg, 4):
                            nb = min(4, Tc - t)
                            q0 = t - gg
                            hs4 = hpool.tile([P, 4, HS], f16, tag="hs")
                            # h-matmuls into 2-bank PSUM pairs so each pair
                            # casts to fp16 in ONE batched op (DVE pair 0,
                            # scalar pair 1) — DVE is per-op-overhead bound
                            for half in range(2):
                                j0 = half * 2
                                if j0 >= nb:
                                    break
                                jn = min(2, nb - j0)
                                hpH = hpsum.tile([P, 2, 512], f32)
                                for j in range(jn):
                                    i = j0 + j
                                    for k in range(KB):
                                        mm(
                                            hpH[:, j, 0:HS],
                                            xe[
                                                :,
                                                k,
                                                (q0 + i) * P : (q0 + i + 1) * P,
                                            ],
                                            Waug_sb[:, k, 0:HS],
                                            start=(k == 0),
                                            stop=(k == KB - 1),
                                        )
                                if half == 0:
                                    nc.vector.tensor_copy(
                                        hs4[:, j0 : j0 + jn, :],
                                        hpH[:, 0:jn, 0:HS],
                                    )
                                else:
                                    nc.scalar.activation(
                                        hs4[:, j0 : j0 + jn, :],
                                        hpH[:, 0:jn, 0:HS],
                                        mybir.ActivationFunctionType.Copy,
                                    )
                            aep4 = apsum.tile([P, 4, HEADS], f32)
                            for i in range(nb):
                                mm(
                                    aep4[:, i, :],
                                    mT_sb[:, t + i, :],
                                    adst_all[:, c, :],
                                    start=True,
                                    stop=True,
                                )
                            e04 = wpool.tile([P, 4, HEADS], f32, tag="e0")
                            nc.vector.tensor_add(
                                e04[:, :nb, :],
                                hs4[:, :nb, IN : IN + HEADS],
                                aep4[:, :nb, :],
                            )
                            # exp(lrelu(e)) == exp(0.2*(e + 4*relu(e)))
                            epos4 = wpool.tile([P, 4, HEADS], f32, tag="ep")
                            nc.scalar.activation(
                                epos4[:, :nb, :],
                                e04[:, :nb, :],
                                mybir.ActivationFunctionType.Relu,
                            )
                            el4 = wpool.tile([P, 4, HEADS], f32, tag="el")
                            nc.vector.scalar_tensor_tensor(
                                out=el4[:, :nb, :],
                                in0=epos4[:, :nb, :],
                                scalar=4.0,
                                in1=e04[:, :nb, :],
                                op0=mybir.AluOpType.mult,
                                op1=mybir.AluOpType.add,
                            )
                            brhs = rpool.tile([P, 4, 4, 65], f16, tag="grhs")
                            nc.scalar.activation(
                                brhs[:, :nb, :, 64],
                                el4[:, :nb, :],
                                mybir.ActivationFunctionType.Exp,
                                scale=NEG_SLOPE,
                            )
                            nbd = min(2, nb)
                            nc.vector.tensor_tensor(
                                out=brhs[:, :nbd, :, 0:64],
                                in0=hs4[:, :nbd, 0:IN].rearrange(
                                    "p q (h c) -> p q h c", h=HEADS
                                ),
                                in1=brhs[:, :nbd, :, 64:65].to_broadcast(
                                    [P, nbd, HEADS, OUT_CH]
                                ),
                                op=mybir.AluOpType.mult,
                            )
                            if nb > nbd:
                                nc.gpsimd.tensor_tensor(
                                    out=brhs[:, nbd:nb, :, 0:64],
                                    in0=hs4[:, nbd:nb, 0:IN].rearrange(
                                        "p q (h c) -> p q h c", h=HEADS
                                    ),
                                    in1=brhs[:, nbd:nb, :, 64:65].to_broadcast(
                                        [P, nb - nbd, HEADS, OUT_CH]
                                    ),
                                    op=mybir.AluOpType.mult,
                                )
                            for i in range(nb):
                                mm(
                                    out_ps[:],
                                    m4_sb[:, t + i, :],
                                    brhs[:, i],
                                    start=(t + i == 0),
                                    stop=(t + i == Tc - 1),
                                )
                    # chunk tail: softmax division, bias, relu, L2 norm
                    dn = fpool.tile([P, HEADS], f32, tag="dn")
                    nc.vector.tensor_scalar_max(dn[:], out_ps[:, :, 64], 1e-30)
                    rdn = fpool.tile([P, HEADS], f32, tag="rdn")
                    nc.vector.reciprocal(rdn[:], dn[:])
                    o1 = fpool.tile([P, HEADS, OUT_CH], f32, tag="o1")
                    nc.vector.tensor_tensor(
                        out=o1[:],
                        in0=out_ps[:, :, 0:64],
                        in1=rdn[:, :, None].to_broadcast([P, HEADS, OUT_CH]),
                        op=mybir.AluOpType.mult,
                    )
                    nc.vector.tensor_add(o1[:], o1[:], bias_full[:])
                    o2 = fpool.tile([P, HEADS, OUT_CH], f32, tag="o2")
                    nc.scalar.activation(
                        o2[:], o1[:], mybir.ActivationFunctionType.Relu
                    )
                    sq = fpool.tile([P, HEADS, OUT_CH], f32, tag="sq")
                    nc.gpsimd.tensor_mul(sq[:], o2[:], o2[:])
                    s = fpool.tile([P, 1], f32, tag="s")
                    nc.vector.tensor_reduce(
                        s[:],
                        sq[:],
                        axis=mybir.AxisListType.XY,
                        op=mybir.AluOpType.add,
                    )
                    r = fpool.tile([P, 1], f32, tag="r")
                    nc.scalar.sqrt(r[:], s[:])
                    nc.vector.tensor_scalar_max(r[:], r[:], 1e-12)
                    rr = fpool.tile([P, 1], f32, tag="rr")
                    nc.vector.reciprocal(rr[:], r[:])
                    o3 = fpool.tile([P, HEADS, OUT_CH], f32, tag="o3")
                    nc.vector.tensor_scalar_mul(o3[:], o2[:], rr[:])
                    nc.sync.dma_start(
                        out=out_d[c * P : c * P + pc, :], in_=o3[:pc]
                    )

    nc.compile()
    return nc


# --------------------------------------------------------------------------
# entry point: full inputs in, full output out
# --------------------------------------------------------------------------
def kernel(x, edge_index, W, att_src, att_dst, bias):
    from concourse.bass_utils import run_bass_kernel_spmd

    n_cores = 8
    meta, in_maps = _preprocess(x, edge_index, W, att_src, att_dst, bias, n_cores)
    nc = _build_program(meta, n_cores)
    res = run_bass_kernel_spmd(nc, in_maps, list(range(n_cores)))
    out = np.concatenate([res.results[k]["out"] for k in range(n_cores)], axis=0)
    return out.astype(np.float32)
